# revision 2
# baseline (speedup 1.0000x reference)
"""Trainium2 Bass kernel for BilinearInteraction.

Reference math (B=2048, F=32 fields, D=64, P=496 field-pairs):
    for pair p=(i,j):  out[b,p,:] = (v_i @ W[p].T) * v_j
    v_i = feature_emb[:, i, :],  v_j = feature_emb[:, j, :]

Sharding: data-parallel over batch, 8 cores x 256 rows each; W replicated.
Output is 260MB fp32 -> the kernel is output-write bound (~32.5MB/core).

Per-core dataflow (all static, Tile-scheduled):
  - W is pre-transposed and packed host-side into wpack[128, 16384]:
    partitions 0:64 hold pairs 0..255 (cols p*64+e = W[p,e,d=partition]),
    partitions 64:128 hold pairs 256..495. One [128,4096] block = one
    64-pair "stage" in each half.
  - featT[64, 8192] = per-field transposed features (col f*256+b), used as
    the stationary matmul operand; duplicated into both partition halves
    on-chip so lhsT/rhs partition ranges always match.
  - featN[256, 2048] = natural-layout features; the elementwise multiplier
    for consecutive pairs of one group is a contiguous slab of it.
  - Per (batch-half bc, stage s): ~9 matmuls [K=64,M=128]x[N<=512] into
    PSUM (pairs of one group chunked by 8), DVE multiply PSUM x featN slab
    into an SBUF staging tile [128, 4096], then one 2MB HWDGE DMA to the
    output row-block (output lands in natural [b, p*64+e] layout).
"""

from itertools import combinations

import numpy as np

N_CORES = 8
B, F, D = 2048, 32, 64
P = 496
B_SH = B // N_CORES            # 256 batch rows per core
STAGE = 64                     # pairs per output stage
N_STAGES = (P + STAGE - 1) // STAGE   # 8 (last stage has 48 pairs)
HALF = 256                     # pair index where the partition half flips

PAIRS = list(combinations(range(F), 2))

_NC_CACHE = {}


def _stage_chunks(s):
    """Chunks of consecutive same-group pairs (<=8) inside stage s."""
    lo, hi = s * STAGE, min((s + 1) * STAGE, P)
    chunks = []
    p = lo
    while p < hi:
        i = PAIRS[p][0]
        e = p
        while e + 1 < hi and PAIRS[e + 1][0] == i and (e + 1 - p) < 8:
            e += 1
        chunks.append((p, e - p + 1))
        p = e + 1
    return chunks


def _build():
    import concourse.tile as tile
    from concourse import bacc, mybir

    F32 = mybir.dt.float32
    nc = bacc.Bacc("TRN2", target_bir_lowering=False, debug=False,
                   enable_asserts=True, num_devices=N_CORES)

    wpack = nc.dram_tensor("wpack", [128, 4 * 4096], F32, kind="ExternalInput").ap()
    featT = nc.dram_tensor("featT", [64, F * B_SH], F32, kind="ExternalInput").ap()
    featN = nc.dram_tensor("featN", [B_SH, F * D], F32, kind="ExternalInput").ap()
    out = nc.dram_tensor("out", [B_SH, P * D], F32, kind="ExternalOutput").ap()

    with tile.TileContext(nc) as tc:
        with (
            tc.tile_pool(name="win", bufs=1) as win,
            tc.tile_pool(name="feat", bufs=1) as feat,
            tc.tile_pool(name="stage", bufs=3) as stage_pool,
            tc.tile_pool(name="psum", bufs=6, space="PSUM") as psum_pool,
        ):
            # resident input tiles ------------------------------------------------
            ft = feat.tile([128, F * B_SH], F32, tag="ft")
            nc.scalar.dma_start(ft[0:64, :], featT[:, :])
            # duplicate into partitions 64:128 (SBUF->SBUF, no HBM traffic)
            nc.scalar.dma_start(ft[64:128, :], ft[0:64, :])

            fn = []
            for bc in range(2):
                t = feat.tile([128, F * D], F32, tag=f"fn{bc}")
                nc.scalar.dma_start(t[:, :], featN[bc * 128:(bc + 1) * 128, :])
                fn.append(t)

            w = []
            for blk in range(4):
                t = win.tile([128, 4096], F32, tag=f"w{blk}")
                nc.scalar.dma_start(t[:, :], wpack[:, blk * 4096:(blk + 1) * 4096])
                w.append(t)

            # compute + output ----------------------------------------------------
            for bc in range(2):
                for s in range(N_STAGES):
                    lo, hi = s * STAGE, min((s + 1) * STAGE, P)
                    po = 0 if s < 4 else 64
                    blk = s % 4
                    st = stage_pool.tile([128, (hi - lo) * D], F32, tag="stage")
                    for (p0, np_) in _stage_chunks(s):
                        i, j0 = PAIRS[p0]
                        ps = psum_pool.tile([128, np_ * D], F32, tag="ps")
                        nc.tensor.matmul(
                            ps[:, :],
                            lhsT=ft[po:po + 64,
                                    i * B_SH + bc * 128: i * B_SH + bc * 128 + 128],
                            rhs=w[blk][po:po + 64,
                                       (p0 - STAGE * s) * D: (p0 - STAGE * s + np_) * D],
                            start=True, stop=True,
                        )
                        nc.vector.tensor_mul(
                            st[:, (p0 - lo) * D: (p0 - lo + np_) * D],
                            ps[:, :],
                            fn[bc][:, j0 * D: (j0 + np_) * D],
                        )
                    nc.sync.dma_start(
                        out[bc * 128: bc * 128 + 128, lo * D: hi * D], st[:, :])
    nc.compile()
    return nc


def _pack_inputs(feature_emb, W):
    feature_emb = np.ascontiguousarray(feature_emb, dtype=np.float32)
    W = np.ascontiguousarray(W, dtype=np.float32)
    Wt = W.transpose(0, 2, 1)                      # [P, d, e]
    wpack = np.zeros((128, 4 * 4096), dtype=np.float32)
    wpack[0:64, :] = Wt[0:HALF].transpose(1, 0, 2).reshape(64, HALF * D)
    wpack[64:128, 0:(P - HALF) * D] = (
        Wt[HALF:P].transpose(1, 0, 2).reshape(64, (P - HALF) * D))
    in_maps = []
    for c in range(N_CORES):
        shard = feature_emb[c * B_SH:(c + 1) * B_SH]         # [256, 32, 64]
        in_maps.append({
            "wpack": wpack,
            "featT": np.ascontiguousarray(
                shard.transpose(2, 1, 0).reshape(64, F * B_SH)),
            "featN": np.ascontiguousarray(shard.reshape(B_SH, F * D)),
        })
    return in_maps


def kernel(feature_emb, W, _trace=False):
    from concourse.bass_utils import run_bass_kernel_spmd

    if "nc" not in _NC_CACHE:
        _NC_CACHE["nc"] = _build()
    nc = _NC_CACHE["nc"]
    in_maps = _pack_inputs(feature_emb, W)
    res = run_bass_kernel_spmd(nc, in_maps, core_ids=list(range(N_CORES)),
                               trace=_trace)
    full = np.concatenate([res.results[c]["out"] for c in range(N_CORES)], axis=0)
    out = full.reshape(B, P, D)
    if _trace:
        return out, res
    return out


# revision 4
# speedup vs baseline: 1.3815x; 1.3815x over previous
"""Trainium2 Bass kernel for BilinearInteraction.

Reference math (B=2048, F=32 fields, D=64, P=496 field-pairs):
    for pair p=(i,j):  out[b,p,:] = (v_i @ W[p].T) * v_j
    v_i = feature_emb[:, i, :],  v_j = feature_emb[:, j, :]

Sharding: data-parallel over batch, 8 cores x 256 rows each; W replicated.
Output is 260MB fp32 -> the kernel is output-write bound (~32.5MB/core).

Per-core dataflow (all static, Tile-scheduled):
  - W is pre-transposed, cast to bf16 and packed host-side into
    wpack[128, 16384]: partitions 0:64 hold pairs 0..255 (cols p*64+e =
    W[p,e,d=partition]), partitions 64:128 hold pairs 256..495. One
    [128,4096] block = one 64-pair "stage" in each half.
  - featT[64, 8192] bf16 = per-field transposed features (col f*256+b),
    the stationary matmul operand; duplicated into both partition halves
    on-chip so lhsT/rhs partition ranges always match.
  - featN[256, 2048] f32 = natural-layout features; the elementwise
    multiplier for consecutive pairs of one group is a contiguous slab.
  - Per (batch-half bc, stage s of 64 pairs): pairs grouped into "runs"
    (same first field, <=16 pairs). Each run: 1-2 matmuls
    [K=64,M=128]x[N<=512] into consecutive PSUM banks of one tile, then
    ONE DVE multiply PSUM x featN slab into the SBUF staging tile.
    Stage completes with a single 2MB HWDGE DMA to the output row-block
    (output lands directly in natural [b, p*64+e] layout).
"""

from itertools import combinations

import numpy as np

N_CORES = 8
B, F, D = 2048, 32, 64
P = 496
B_SH = B // N_CORES            # 256 batch rows per core
STAGE = 64                     # pairs per output stage
N_STAGES = (P + STAGE - 1) // STAGE   # 8 (last stage has 48 pairs)
HALF = 256                     # pair index where the partition half flips
RUN = 16                       # max pairs per DVE multiply (2 PSUM banks)

PAIRS = list(combinations(range(F), 2))

_NC_CACHE = {}


def _stage_runs(s):
    """Runs of consecutive same-group pairs (<=RUN) inside stage s."""
    lo, hi = s * STAGE, min((s + 1) * STAGE, P)
    runs = []
    p = lo
    while p < hi:
        i = PAIRS[p][0]
        e = p
        while e + 1 < hi and PAIRS[e + 1][0] == i and (e + 1 - p) < RUN:
            e += 1
        runs.append((p, e - p + 1))
        p = e + 1
    return runs


def _build():
    import concourse.tile as tile
    from concourse import bacc, mybir

    F32 = mybir.dt.float32
    BF16 = mybir.dt.bfloat16
    nc = bacc.Bacc("TRN2", target_bir_lowering=False, debug=False,
                   enable_asserts=True, num_devices=N_CORES)

    wpack = nc.dram_tensor("wpack", [128, 4 * 4096], BF16, kind="ExternalInput").ap()
    featT = nc.dram_tensor("featT", [64, F * B_SH], BF16, kind="ExternalInput").ap()
    featN = nc.dram_tensor("featN", [B_SH, F * D], F32, kind="ExternalInput").ap()
    out = nc.dram_tensor("out", [B_SH, P * D], F32, kind="ExternalOutput").ap()

    with tile.TileContext(nc) as tc:
        with (
            tc.tile_pool(name="win", bufs=1) as win,
            tc.tile_pool(name="feat", bufs=1) as feat,
            tc.tile_pool(name="stage", bufs=3) as stage_pool,
            tc.tile_pool(name="psum", bufs=4, space="PSUM") as psum_pool,
        ):
            # resident input tiles, in the order stage 0 needs them --------------
            w = [win.tile([128, 4096], BF16, name=f"w{blk}", tag=f"w{blk}")
                 for blk in range(4)]
            ft = feat.tile([128, F * B_SH], BF16, name="ft", tag="ft")
            fn = [feat.tile([128, F * D], F32, name=f"fn{bc}", tag=f"fn{bc}")
                  for bc in range(2)]

            nc.scalar.dma_start(w[0][:, :], wpack[:, 0:4096])
            nc.scalar.dma_start(fn[0][:, :], featN[0:128, :])
            nc.scalar.dma_start(ft[0:64, :], featT[:, :])
            # duplicate into partitions 64:128 (SBUF->SBUF, no HBM traffic)
            nc.scalar.dma_start(ft[64:128, :], ft[0:64, :])
            nc.scalar.dma_start(w[1][:, :], wpack[:, 4096:8192])
            nc.scalar.dma_start(fn[1][:, :], featN[128:256, :])
            nc.scalar.dma_start(w[2][:, :], wpack[:, 8192:12288])
            nc.scalar.dma_start(w[3][:, :], wpack[:, 12288:16384])

            # compute + output ----------------------------------------------------
            for bc in range(2):
                for s in range(N_STAGES):
                    lo, hi = s * STAGE, min((s + 1) * STAGE, P)
                    po = 0 if s < 4 else 64
                    blk = s % 4
                    st = stage_pool.tile([128, (hi - lo) * D], F32, tag="stage")
                    for (p0, n) in _stage_runs(s):
                        i, j0 = PAIRS[p0]
                        ps = psum_pool.tile([128, RUN * D], F32, tag="ps")
                        for k in range(0, n, 8):
                            nk = min(8, n - k)
                            nc.tensor.matmul(
                                ps[:, k * D:(k + nk) * D],
                                lhsT=ft[po:po + 64,
                                        i * B_SH + bc * 128:
                                        i * B_SH + bc * 128 + 128],
                                rhs=w[blk][po:po + 64,
                                           (p0 + k - STAGE * s) * D:
                                           (p0 + k + nk - STAGE * s) * D],
                                start=True, stop=True,
                            )
                        nc.vector.tensor_mul(
                            st[:, (p0 - lo) * D: (p0 - lo + n) * D],
                            ps[:, 0:n * D],
                            fn[bc][:, j0 * D: (j0 + n) * D],
                        )
                    nc.sync.dma_start(
                        out[bc * 128: bc * 128 + 128, lo * D: hi * D], st[:, :])
    nc.compile()
    return nc


def _pack_inputs(feature_emb, W):
    import ml_dtypes

    BF = ml_dtypes.bfloat16
    feature_emb = np.ascontiguousarray(feature_emb, dtype=np.float32)
    W = np.ascontiguousarray(W, dtype=np.float32)
    Wt = W.transpose(0, 2, 1)                      # [P, d, e]
    wpack = np.zeros((128, 4 * 4096), dtype=BF)
    wpack[0:64, :] = Wt[0:HALF].transpose(1, 0, 2).reshape(64, HALF * D).astype(BF)
    wpack[64:128, 0:(P - HALF) * D] = (
        Wt[HALF:P].transpose(1, 0, 2).reshape(64, (P - HALF) * D).astype(BF))
    in_maps = []
    for c in range(N_CORES):
        shard = feature_emb[c * B_SH:(c + 1) * B_SH]         # [256, 32, 64]
        in_maps.append({
            "wpack": wpack,
            "featT": np.ascontiguousarray(
                shard.transpose(2, 1, 0).reshape(64, F * B_SH).astype(BF)),
            "featN": np.ascontiguousarray(shard.reshape(B_SH, F * D)),
        })
    return in_maps


def kernel(feature_emb, W, _trace=False):
    from concourse.bass_utils import run_bass_kernel_spmd

    if "nc" not in _NC_CACHE:
        _NC_CACHE["nc"] = _build()
    nc = _NC_CACHE["nc"]
    in_maps = _pack_inputs(feature_emb, W)
    res = run_bass_kernel_spmd(nc, in_maps, core_ids=list(range(N_CORES)),
                               trace=_trace)
    full = np.concatenate([res.results[c]["out"] for c in range(N_CORES)], axis=0)
    out = full.reshape(B, P, D)
    if _trace:
        return out, res
    return out


# revision 5
# speedup vs baseline: 1.8959x; 1.3723x over previous
"""Trainium2 Bass kernel for BilinearInteraction.

Reference math (B=2048, F=32 fields, D=64, P=496 field-pairs):
    for pair p=(i,j):  out[b,p,:] = (v_i @ W[p].T) * v_j
    v_i = feature_emb[:, i, :],  v_j = feature_emb[:, j, :]

Sharding: data-parallel over batch, 8 cores x 256 rows each; W replicated.
The fp32 output is 260MB (32.5MB/core) -> the kernel is output-write bound,
so the device writes bf16 (16.25MB/core) and the host upcasts; combined with
bf16 matmul operands the end-to-end relative error is ~3e-3, well inside the
2e-2 gate.

Per-core dataflow (all static, Tile-scheduled):
  - W is pre-transposed, cast to bf16 and packed host-side into
    wpack[128, 16384]: partitions 0:64 hold pairs 0..255 (cols p*64+e =
    W[p,e,d=partition]), partitions 64:128 hold pairs 256..495. One
    [128,4096] block = one 64-pair "stage" in each half.
  - featT[64, 8192] bf16 = per-field transposed features (col f*256+b),
    the stationary matmul operand; duplicated into both partition halves
    on-chip so lhsT/rhs base partitions match (a matmul requirement).
  - featN[256, 2048] f32 = natural-layout features; the elementwise
    multiplier for consecutive pairs of one group is a contiguous slab.
  - Per (batch-half bc, stage s of 64 pairs): pairs grouped into "runs"
    (same first field, <=16 pairs). Each run: 1-2 matmuls
    [K=64,M=128]x[N<=512] into consecutive PSUM banks of one tile, then
    the PSUM x featN Hadamard product via one of two engine paths chosen
    to balance load (DVE TT from PSUM runs at ~95 elem/ns; GpSimd cannot
    read PSUM, so its path is ACT copy PSUM->SBUF f32 at ~95 then GpSimd
    TT at ~56; ACT/GpSimd are otherwise idle):
       path A (~63%): DVE  tensor_mul(psum_f32, featN_f32) -> stage bf16
       path C (~37%): ACT  copy psum -> tmp f32;
                      GPS  tensor_mul(tmp, featN_f32)      -> stage bf16
    Stage completes with a single 1MB HWDGE DMA to the output row-block
    (output lands directly in natural [b, p*64+e] layout).
"""

from itertools import combinations

import numpy as np

N_CORES = 8
B, F, D = 2048, 32, 64
P = 496
B_SH = B // N_CORES            # 256 batch rows per core
STAGE = 64                     # pairs per output stage
N_STAGES = (P + STAGE - 1) // STAGE   # 8 (last stage has 48 pairs)
HALF = 256                     # pair index where the partition half flips
RUN = 16                       # max pairs per Hadamard op (2 PSUM banks)
GPS_FRAC = 0.37                # share of elements routed to the GpSimd path

PAIRS = list(combinations(range(F), 2))

_NC_CACHE = {}


def _stage_runs(s):
    """Runs of consecutive same-group pairs (<=RUN) inside stage s."""
    lo, hi = s * STAGE, min((s + 1) * STAGE, P)
    runs = []
    p = lo
    while p < hi:
        i = PAIRS[p][0]
        e = p
        while e + 1 < hi and PAIRS[e + 1][0] == i and (e + 1 - p) < RUN:
            e += 1
        runs.append((p, e - p + 1))
        p = e + 1
    return runs


def _build():
    import concourse.tile as tile
    from concourse import bacc, mybir

    F32 = mybir.dt.float32
    BF16 = mybir.dt.bfloat16
    nc = bacc.Bacc("TRN2", target_bir_lowering=False, debug=False,
                   enable_asserts=True, num_devices=N_CORES)

    wpack = nc.dram_tensor("wpack", [128, 4 * 4096], BF16, kind="ExternalInput").ap()
    featT = nc.dram_tensor("featT", [64, F * B_SH], BF16, kind="ExternalInput").ap()
    featN = nc.dram_tensor("featN", [B_SH, F * D], F32, kind="ExternalInput").ap()
    out = nc.dram_tensor("out", [B_SH, P * D], BF16, kind="ExternalOutput").ap()

    with tile.TileContext(nc) as tc:
        with (
            tc.tile_pool(name="win", bufs=1) as win,
            tc.tile_pool(name="feat", bufs=1) as feat,
            tc.tile_pool(name="stage", bufs=3) as stage_pool,
            tc.tile_pool(name="tmp", bufs=3) as tmp_pool,
            tc.tile_pool(name="psum", bufs=4, space="PSUM") as psum_pool,
        ):
            # resident input tiles, in the order stage 0 needs them --------------
            w = [win.tile([128, 4096], BF16, name=f"w{blk}", tag=f"w{blk}")
                 for blk in range(4)]
            ft = feat.tile([128, F * B_SH], BF16, name="ft", tag="ft")
            fn = [feat.tile([128, F * D], F32, name=f"fn{bc}", tag=f"fn{bc}")
                  for bc in range(2)]

            nc.scalar.dma_start(w[0][:, :], wpack[:, 0:4096])
            nc.scalar.dma_start(fn[0][:, :], featN[0:128, :])
            nc.scalar.dma_start(ft[0:64, :], featT[:, :])
            # duplicate into partitions 64:128 (SBUF->SBUF, no HBM traffic)
            nc.scalar.dma_start(ft[64:128, :], ft[0:64, :])
            nc.scalar.dma_start(w[1][:, :], wpack[:, 4096:8192])
            nc.scalar.dma_start(fn[1][:, :], featN[128:256, :])
            nc.scalar.dma_start(w[2][:, :], wpack[:, 8192:12288])
            nc.scalar.dma_start(w[3][:, :], wpack[:, 12288:16384])

            # compute + output ----------------------------------------------------
            el_tot = el_gps = 0
            for bc in range(2):
                for s in range(N_STAGES):
                    lo, hi = s * STAGE, min((s + 1) * STAGE, P)
                    po = 0 if s < 4 else 64
                    blk = s % 4
                    st = stage_pool.tile([128, (hi - lo) * D], BF16, tag="stage")
                    for (p0, n) in _stage_runs(s):
                        i, j0 = PAIRS[p0]
                        ps = psum_pool.tile([128, RUN * D], F32, tag="ps")
                        for k in range(0, n, 8):
                            nk = min(8, n - k)
                            nc.tensor.matmul(
                                ps[:, k * D:(k + nk) * D],
                                lhsT=ft[po:po + 64,
                                        i * B_SH + bc * 128:
                                        i * B_SH + bc * 128 + 128],
                                rhs=w[blk][po:po + 64,
                                           (p0 + k - STAGE * s) * D:
                                           (p0 + k + nk - STAGE * s) * D],
                                start=True, stop=True,
                            )
                        st_sl = st[:, (p0 - lo) * D: (p0 - lo + n) * D]
                        fn_sl = fn[bc][:, j0 * D: (j0 + n) * D]
                        el_tot += n
                        if el_gps < GPS_FRAC * el_tot:
                            el_gps += n
                            tmp = tmp_pool.tile([128, RUN * D], F32, tag="tmp")
                            nc.scalar.copy(tmp[:, 0:n * D], ps[:, 0:n * D])
                            nc.gpsimd.tensor_mul(st_sl, tmp[:, 0:n * D], fn_sl)
                        else:
                            nc.vector.tensor_mul(st_sl, ps[:, 0:n * D], fn_sl)
                    nc.sync.dma_start(
                        out[bc * 128: bc * 128 + 128, lo * D: hi * D], st[:, :])
    nc.compile()
    return nc


def _pack_inputs(feature_emb, W):
    import ml_dtypes

    BF = ml_dtypes.bfloat16
    feature_emb = np.ascontiguousarray(feature_emb, dtype=np.float32)
    W = np.ascontiguousarray(W, dtype=np.float32)
    Wt = W.transpose(0, 2, 1)                      # [P, d, e]
    wpack = np.zeros((128, 4 * 4096), dtype=BF)
    wpack[0:64, :] = Wt[0:HALF].transpose(1, 0, 2).reshape(64, HALF * D).astype(BF)
    wpack[64:128, 0:(P - HALF) * D] = (
        Wt[HALF:P].transpose(1, 0, 2).reshape(64, (P - HALF) * D).astype(BF))
    in_maps = []
    for c in range(N_CORES):
        shard = feature_emb[c * B_SH:(c + 1) * B_SH]         # [256, 32, 64]
        in_maps.append({
            "wpack": wpack,
            "featT": np.ascontiguousarray(
                shard.transpose(2, 1, 0).reshape(64, F * B_SH).astype(BF)),
            "featN": np.ascontiguousarray(shard.reshape(B_SH, F * D)),
        })
    return in_maps


def kernel(feature_emb, W, _trace=False):
    from concourse.bass_utils import run_bass_kernel_spmd

    if "nc" not in _NC_CACHE:
        _NC_CACHE["nc"] = _build()
    nc = _NC_CACHE["nc"]
    in_maps = _pack_inputs(feature_emb, W)
    res = run_bass_kernel_spmd(nc, in_maps, core_ids=list(range(N_CORES)),
                               trace=_trace)
    full = np.concatenate(
        [res.results[c]["out"].astype(np.float32) for c in range(N_CORES)], axis=0)
    out = full.reshape(B, P, D)
    if _trace:
        return out, res
    return out
